# revision 14
# baseline (speedup 1.0000x reference)
"""Distributed Trainium2 kernel for a dense transformer block.

Reference computation (per batch):
  x = x + o_proj(attn(rope(qkv(rmsnorm(x))), causal)) ; x = x + w2(silu(wg(rmsnorm(x))) * w1(rmsnorm(x)))

Sharding v2: DP=2 on batch x TP=4 on heads for attention; the o-proj
partial sums are combined with two chunked ReduceScatters (tokens
0-1023 and 1024-2047), after which each core owns 512 tokens and runs
the FULL MLP on them sequence-parallel (weights streamed from HBM), so
there is no second collective at all.  The final [512,1024] shards are
assembled on the host.
"""

import sys

sys.path.insert(0, "/opt/trn_rl_repo")

import numpy as np
import ml_dtypes

import concourse.bass as bass
import concourse.bacc as bacc
import concourse.mybir as mybir
import concourse.tile as tile
from concourse.bass_utils import run_bass_kernel_spmd

BF = ml_dtypes.bfloat16
F32 = mybir.dt.float32
BF16 = mybir.dt.bfloat16

D = 1024
NH = 16
DH = 64
MULT = 4
EPS = 1e-5
ROPE_BASE = 10000.0
B = 2
TP = 4  # tensor-parallel ranks per group
HPC = NH // TP  # heads per core = 4
QKF = 2 * HPC * DH  # q+k shard features = 512
VF = HPC * DH  # v shard features = 256
MIDF = MULT * D  # full mlp rows = 4096
AF = mybir.ActivationFunctionType
ALU = mybir.AluOpType


def build_nc(T):
    """Build the SPMD graph for one core (token count T per batch)."""
    assert T == 2048
    DC = D // 128  # d chunks = 8
    TT = T // 128  # token tiles = 16
    QT = 512  # q-tile width
    NQ = T // QT  # 4
    CPQ = QT // 128  # 128-token tiles per q chunk = 4
    NT = D // 512  # 2
    MFC = MIDF // 128  # mlp row chunks = 32
    TOK = T // TP  # owned tokens per core = 512
    WT = TOK // 2  # tokens per MLP wave = 256
    WTT = WT // 128  # token tiles per wave = 2

    nc = bacc.Bacc("TRN2", target_bir_lowering=False, debug=False, num_devices=8)

    x_e = nc.dram_tensor("x", [T, D], F32, kind="ExternalInput")
    qkw_e = nc.dram_tensor("qkw_t", [D, QKF], BF16, kind="ExternalInput")
    vw_e = nc.dram_tensor("vw_m", [D, VF], BF16, kind="ExternalInput")
    ow_e = nc.dram_tensor("ow_m", [VF, D], BF16, kind="ExternalInput")
    # packed [128, MFC, DC*128]: (p, mc, c*128+m) with d = c*128+p
    w1w_e = nc.dram_tensor("w1w_p", [128, MFC, DC * 128], BF16, kind="ExternalInput")
    wgw_e = nc.dram_tensor("wgw_p", [128, MFC, DC * 128], BF16, kind="ExternalInput")
    w2w_e = nc.dram_tensor("w2w_m", [MIDF, D], BF16, kind="ExternalInput")
    cos_e = nc.dram_tensor("cosr", [128, T], BF16, kind="ExternalInput")
    sin_e = nc.dram_tensor("sinr", [128, T], BF16, kind="ExternalInput")
    cm_e = nc.dram_tensor("cmask", [CPQ * 128, QT], BF16, kind="ExternalInput")
    id_e = nc.dram_tensor("ident", [128, 128], BF16, kind="ExternalInput")
    out_e = nc.dram_tensor("out", [TOK, D], F32, kind="ExternalOutput")

    groups = [[0, 1, 2, 3], [4, 5, 6, 7]]

    with tile.TileContext(nc) as tc:
        with (
            tc.tile_pool(name="const", bufs=1) as cpool,
            tc.tile_pool(name="actfm", bufs=1) as fmpool,
            tc.tile_pool(name="qko", bufs=1) as qkpool,
            tc.tile_pool(name="vaug", bufs=1) as vpool,
            tc.tile_pool(name="xin", bufs=2) as xpool,
            tc.tile_pool(name="xnb", bufs=3) as xnpool,
            tc.tile_pool(name="work", bufs=4) as wpool,
            tc.tile_pool(name="rope", bufs=2) as rpool,
            tc.tile_pool(name="stats", bufs=8) as spool,
            tc.tile_pool(name="mlpw", bufs=3) as mwpool,
            tc.tile_pool(name="mlpa", bufs=1) as apool,
            tc.tile_pool(name="psA", bufs=4, space="PSUM") as psA,
            tc.tile_pool(name="psO", bufs=2, space="PSUM") as psO,
            tc.tile_pool(name="psS", bufs=2, space="PSUM") as psS,
            tc.tile_pool(name="dram", bufs=1, space="DRAM") as dpool,
        ):
            # ---- resident weights / tables ----
            def load_tiles(src, width, n, dt=BF16):
                ts = []
                for i in range(n):
                    t = cpool.tile(
                        [128, width], dt, tag=f"{src.name}_{i}", name=f"{src.name}_{i}"
                    )
                    nc.sync.dma_start(t[:], src[i * 128 : (i + 1) * 128, :])
                    ts.append(t)
                return ts

            qkw = load_tiles(qkw_e, QKF, DC)
            vw = load_tiles(vw_e, VF, DC)
            ow = load_tiles(ow_e, D, VF // 128)
            cosr = load_tiles(cos_e, T, 1)[0]
            sinr = load_tiles(sin_e, T, 1)[0]
            cmask = load_tiles(cm_e, QT, CPQ)
            ones64 = cpool.tile([1, 64], BF16, tag="ones64", name="ones64")
            nc.vector.memset(ones64[:], 1.0)
            ident = load_tiles(id_e, 128, 1)[0]
            epsc = cpool.tile([128, 1], F32, tag="epsc", name="epsc")
            nc.vector.memset(epsc[:], EPS)

            rs_in = dpool.tile([T, D], BF16, name="rs_in")
            rs_out = dpool.tile([TOK, D], BF16, name="rs_out")

            # ---- persistent activation tiles ----
            xnf = [
                fmpool.tile([128, T], BF16, tag=f"fm{d}", name=f"xnf{d}")
                for d in range(DC)
            ]
            q_sb = [
                qkpool.tile([128, T], BF16, tag=f"qk{i}", name=f"q{i}")
                for i in range(2)
            ]
            k_sb = [
                qkpool.tile([128, T], BF16, tag=f"qk{i + 2}", name=f"k{i}")
                for i in range(2)
            ]
            O_sb = [
                qkpool.tile([128, T], BF16, tag=f"qk{i + 4}", name=f"O{i}")
                for i in range(2)
            ]
            v_aug = [
                vpool.tile([128, HPC, DH + 1], BF16, tag=f"va{ti}", name=f"va{ti}")
                for ti in range(TT)
            ]

            # ---- helpers ----
            def norm_into_fm(xt, fm_tiles, ti):
                """rmsnorm the token tile xt, write bf16 feature-major columns
                [ti*128, (ti+1)*128) of each fm tile."""
                ss = spool.tile([128, 1], F32, tag="ss", name="ss")
                sq = xnpool.tile([128, D], BF16, tag="sq", name="sq", bufs=2)
                nc.scalar.activation(
                    out=sq[:], in_=xt[:], func=AF.Square, accum_out=ss[:]
                )
                sr = spool.tile([128, 1], F32, tag="sr", name="sr")
                nc.scalar.activation(
                    out=sr[:], in_=ss[:], func=AF.Sqrt, bias=epsc[:], scale=1.0 / D
                )
                s1 = spool.tile([128, 1], F32, tag="s1", name="s1")
                nc.vector.reciprocal(s1[:], sr[:])
                xn = xnpool.tile([128, D], BF16, tag="xn", name="xn")
                nc.vector.tensor_scalar_mul(xn[:], xt[:], s1[:])
                for di in range(DC):
                    tp = psS.tile([128, 128], BF16, tag="tp", name="tp", bufs=2)
                    nc.tensor.transpose(
                        tp[:], xn[:, di * 128 : (di + 1) * 128], ident[:]
                    )
                    nc.any.tensor_copy(
                        fm_tiles[di][:, ti * 128 : (ti + 1) * 128], tp[:]
                    )

            # ---- stage A: norm1 + transpose ----
            for ti in range(TT):
                xt = xpool.tile([128, D], F32, tag="xt", name="xt")
                nc.sync.dma_start(xt[:], x_e[ti * 128 : (ti + 1) * 128, :])
                norm_into_fm(xt, xnf, ti)

            # ---- stage B: qkv + rope (unit-granular for filler pumping) ----
            def qk_unit(t4, m):  # m in 0..3: q01 q23 k01 k23
                tsl = slice(t4 * QT, (t4 + 1) * QT)
                dst = q_sb[m] if m < 2 else k_sb[m - 2]
                ps = psA.tile([128, 512], F32, tag="ps", name="ps")
                for dc in range(DC):
                    nc.tensor.matmul(
                        ps[:, :QT],
                        qkw[dc][:, m * 128 : (m + 1) * 128],
                        xnf[dc][:, tsl],
                        start=(dc == 0),
                        stop=(dc == DC - 1),
                    )
                qb = rpool.tile([128, QT], BF16, tag="qb", name="qb")
                nc.vector.tensor_copy(qb[:], ps[:, :QT])
                rot = rpool.tile([128, QT], BF16, tag="rot", name="rot")
                for hb in (0, 64):
                    nc.vector.tensor_scalar_mul(
                        rot[hb : hb + 32, :], qb[hb + 32 : hb + 64, :], -1.0
                    )
                    nc.vector.tensor_copy(
                        rot[hb + 32 : hb + 64, :], qb[hb : hb + 32, :]
                    )
                t1 = rpool.tile([128, QT], BF16, tag="t1", name="t1")
                nc.vector.tensor_mul(t1[:], qb[:], cosr[:, tsl])
                t2 = rpool.tile([128, QT], BF16, tag="t2", name="t2")
                nc.vector.tensor_mul(t2[:], rot[:], sinr[:, tsl])
                nc.vector.tensor_add(dst[:, tsl], t1[:], t2[:])

            def v_unit(ti):
                ps = psS.tile([128, VF], F32, tag="tp", name="psv")
                for dc in range(DC):
                    nc.tensor.matmul(
                        ps[:],
                        xnf[dc][:, ti * 128 : (ti + 1) * 128],
                        vw[dc][:],
                        start=(dc == 0),
                        stop=(dc == DC - 1),
                    )
                va = v_aug[ti]
                nc.vector.tensor_copy(
                    va[:, :, 0:DH], ps.rearrange("p (h d) -> p h d", h=HPC)
                )
                nc.gpsimd.memset(va[:, :, DH : DH + 1], 1.0)

            def qkv_fillers(t4):
                fs = [lambda m=m: qk_unit(t4, m) for m in range(4)]
                fs += [lambda ti=ti: v_unit(ti) for ti in range(t4 * CPQ, (t4 + 1) * CPQ)]
                return fs

            # ---- stage C: attention (with filler pump to keep PE dense) ----
            def attn_qtile(qt, fillers):
                tsl = slice(qt * QT, (qt + 1) * QT)
                ncks = CPQ * (qt + 1)
                nsteps = 2 * ncks
                pump_every = max(1, nsteps // max(1, len(fillers))) if fillers else 0
                step = 0
                rinvb = spool.tile(
                    [1, HPC * QT], BF16, tag="rinvb", name="rinvb", bufs=2
                )
                for hp in range(2):
                    opsP = [
                        psO.tile([DH + 1, QT], F32, tag="pso", name=f"ops{i}")
                        for i in range(2)
                    ]
                    for ck in range(ncks):
                        j = ck - CPQ * qt
                        lo = j * 128 if j > 0 else 0
                        # both head scores back-to-back -> PE row-tile packing
                        sps = []
                        for i in range(2):
                            hb = i * 64
                            sp = psA.tile([128, 512], F32, tag="ps", name="sp")
                            nc.tensor.matmul(
                                sp[:, lo:QT],
                                k_sb[hp][hb : hb + DH, ck * 128 : (ck + 1) * 128],
                                q_sb[hp][hb : hb + DH, qt * QT + lo : (qt + 1) * QT],
                                start=True,
                                stop=True,
                            )
                            sps.append(sp)
                        pts = []
                        for i in range(2):
                            sp = sps[i]
                            pt = wpool.tile(
                                [128, QT], BF16, tag="pt", name="pt", bufs=6
                            )
                            nc.scalar.activation(
                                out=pt[:, lo:],
                                in_=sp[:, lo:QT],
                                func=AF.Exp,
                                scale=0.125,
                            )
                            if 0 <= j:
                                # mask only the 128-wide diagonal band
                                nc.gpsimd.tensor_mul(
                                    pt[:, lo : lo + 128],
                                    pt[:, lo : lo + 128],
                                    cmask[j][:, lo : lo + 128],
                                )
                            pts.append(pt)
                        for i in range(2):
                            nc.tensor.matmul(
                                opsP[i][:, lo:],
                                v_aug[ck][:, 2 * hp + i, :],
                                pts[i][:, lo:],
                                start=(ck == 0),
                                stop=(ck == ncks - 1),
                            )
                        step += 1
                        if fillers and pump_every and step % pump_every == 0:
                            fillers.pop(0)()
                    for i in range(2):
                        h = 2 * hp + i
                        ops = opsP[i]
                        rsc = spool.tile([1, QT], F32, tag="rsc", name="rsc", bufs=2)
                        nc.vector.reciprocal(rsc[:], ops[DH : DH + 1, :])
                        nc.vector.tensor_copy(
                            rinvb[:, h * QT : (h + 1) * QT], rsc[:]
                        )
                        nc.vector.tensor_copy(
                            O_sb[hp][i * 64 : i * 64 + DH, tsl], ops[0:DH, :]
                        )
                while fillers:
                    fillers.pop(0)()
                return rinvb

            def normalize_qt(qt, rinvb):
                tsl = slice(qt * QT, (qt + 1) * QT)
                for ot in range(2):
                    bb = psA.tile([128, 512], F32, tag="ps", name="bb")
                    for i in range(2):
                        h = 2 * ot + i
                        nc.tensor.matmul(
                            bb[i * 64 : (i + 1) * 64, :QT],
                            ones64[:],
                            rinvb[:, h * QT : (h + 1) * QT],
                            start=True,
                            stop=True,
                        )
                    nc.vector.tensor_mul(
                        O_sb[ot][:, tsl], O_sb[ot][:, tsl], bb[:, :QT]
                    )

            def oproj_rs(qt):
                for ti in range(qt * CPQ, (qt + 1) * CPQ):
                    ob = wpool.tile([128, D], BF16, tag="ob", name="ob", bufs=3)
                    xo = xpool.tile([128, D], F32, tag="xo", name="xo")
                    nc.sync.dma_start(xo[:], x_e[ti * 128 : (ti + 1) * 128, :])
                    for nt in range(NT):
                        ps = psA.tile([128, 512], F32, tag="ps", name="ps")
                        for c in range(VF // 128):
                            nc.tensor.matmul(
                                ps[:, :512],
                                O_sb[c][:, ti * 128 : (ti + 1) * 128],
                                ow[c][:, nt * 512 : (nt + 1) * 512],
                                start=(c == 0),
                                stop=(c == VF // 128 - 1),
                            )
                        nc.vector.scalar_tensor_tensor(
                            ob[:, nt * 512 : (nt + 1) * 512],
                            xo[:, nt * 512 : (nt + 1) * 512],
                            1.0 / TP,
                            ps[:, :512],
                            ALU.mult,
                            ALU.add,
                        )
                    nc.sync.dma_start(rs_in[ti * 128 : (ti + 1) * 128, :], ob[:])

            def rs_fire(half):
                lo, hi = half * (T // 2), (half + 1) * (T // 2)
                olo, ohi = half * WT, (half + 1) * WT
                nc.gpsimd.collective_compute(
                    "ReduceScatter",
                    ALU.add,
                    ins=[rs_in[lo:hi, :].opt()],
                    outs=[rs_out[olo:ohi, :].opt()],
                    replica_groups=groups,
                )

            # ---- stage D: sequence-parallel MLP on own 512 tokens ----
            hnf = [
                fmpool.tile([128, WT], BF16, tag=f"hf{d}", name=f"hnf{d}")
                for d in range(DC)
            ]
            a_sb = [
                apool.tile([128, WT], BF16, tag=f"a{m}", name=f"a{m}")
                for m in range(MFC)
            ]
            h1_tiles = {}

            def wave_prep(w):
                h1t = []
                for tt in range(WTT):
                    h1 = xpool.tile([128, D], BF16, tag=f"h1_{tt}", name="h1", bufs=2)
                    nc.gpsimd.dma_start(
                        h1[:], rs_out[w * WT + tt * 128 : w * WT + (tt + 1) * 128, :]
                    )
                    h1t.append(h1)
                    norm_into_fm(h1, hnf, tt)
                h1_tiles[w] = h1t

            def split_dma(dst, src):
                nc.sync.dma_start(dst[0:64, :], src[0:64, :])
                nc.sync.dma_start(dst[64:128, :], src[64:128, :])

            def mc_unit(mc):
                # pass 1 for one m-chunk: a = silu(x@wgT) * (x@w1T)
                wg_mc = mwpool.tile([128, DC * 128], BF16, tag="wgs", name="wg_mc")
                split_dma(wg_mc, wgw_e[:, mc, :])
                w1_mc = mwpool.tile([128, DC * 128], BF16, tag="w1s", name="w1_mc")
                split_dma(w1_mc, w1w_e[:, mc, :])
                psg = psA.tile([128, 512], F32, tag="ps", name="psg")
                for dc in range(DC):
                    nc.tensor.matmul(
                        psg[:, :WT],
                        wg_mc[:, dc * 128 : (dc + 1) * 128],
                        hnf[dc][:],
                        start=(dc == 0),
                        stop=(dc == DC - 1),
                    )
                g_sb = wpool.tile([128, WT], BF16, tag="g", name="g", bufs=2)
                nc.scalar.activation(out=g_sb[:], in_=psg[:, :WT], func=AF.Silu)
                psu = psA.tile([128, 512], F32, tag="ps", name="psu")
                for dc in range(DC):
                    nc.tensor.matmul(
                        psu[:, :WT],
                        w1_mc[:, dc * 128 : (dc + 1) * 128],
                        hnf[dc][:],
                        start=(dc == 0),
                        stop=(dc == DC - 1),
                    )
                nc.vector.tensor_mul(a_sb[mc][:], g_sb[:], psu[:, :WT])

            def pass2(w):
                # out = h1 + a@w2T
                for nt in range(NT):
                    pws = [
                        psA.tile([128, 512], F32, tag="ps", name=f"pw{tt}")
                        for tt in range(WTT)
                    ]
                    for mc in range(MFC):
                        w2_mc = mwpool.tile(
                            [128, 512], BF16, tag="w2s", name="w2_mc", bufs=6
                        )
                        split_dma(
                            w2_mc,
                            w2w_e[
                                mc * 128 : (mc + 1) * 128,
                                nt * 512 : (nt + 1) * 512,
                            ],
                        )
                        for tt in range(WTT):
                            nc.tensor.matmul(
                                pws[tt][:],
                                a_sb[mc][:, tt * 128 : (tt + 1) * 128],
                                w2_mc[:],
                                start=(mc == 0),
                                stop=(mc == MFC - 1),
                            )
                    for tt in range(WTT):
                        ot = wpool.tile(
                            [128, 512], F32, tag="otile", name="ot", bufs=2
                        )
                        nc.vector.scalar_tensor_tensor(
                            ot[:],
                            h1_tiles[w][tt][:, nt * 512 : (nt + 1) * 512],
                            1.0,
                            pws[tt][:],
                            ALU.mult,
                            ALU.add,
                        )
                        nc.sync.dma_start(
                            out_e[
                                w * WT + tt * 128 : w * WT + (tt + 1) * 128,
                                nt * 512 : (nt + 1) * 512,
                            ],
                            ot[:],
                        )

            # ---- phase schedule ----
            for f in qkv_fillers(0):
                f()
            rinvb = attn_qtile(0, qkv_fillers(1))
            normalize_qt(0, rinvb)
            oproj_rs(0)
            rinvb = attn_qtile(1, qkv_fillers(2))
            normalize_qt(1, rinvb)
            oproj_rs(1)
            rs_fire(0)
            rinvb = attn_qtile(2, qkv_fillers(3))
            normalize_qt(2, rinvb)
            oproj_rs(2)
            # wave-0 MLP prep + first half of pass 1 pumped into the longest q-tile
            w0_fill = [lambda: wave_prep(0)]
            w0_fill += [lambda mc=mc: mc_unit(mc) for mc in range(16)]
            rinvb = attn_qtile(3, w0_fill)
            normalize_qt(3, rinvb)
            oproj_rs(3)
            rs_fire(1)
            for mc in range(16, MFC):
                mc_unit(mc)
            pass2(0)
            wave_prep(1)
            for mc in range(MFC):
                mc_unit(mc)
            pass2(1)

    nc.compile()
    return nc


def make_in_maps(x, n1_w, n2_w, qkv_w, o_w, w1_w, wg_w, w2_w, T):
    QT = 512
    CPQ = QT // 128
    half = DH // 2
    freqs = np.arange(half, dtype=np.float64) / half
    theta = 1.0 / ROPE_BASE**freqs
    ang = np.arange(T, dtype=np.float64)[:, None] * theta[None, :]  # [T, 32]
    p = np.arange(128) % half
    cosr = np.cos(ang)[:, p].T.astype(BF)  # [128, T]
    sinr = np.sin(ang)[:, p].T.astype(BF)
    cm = np.zeros((CPQ * 128, QT), dtype=BF)
    for j in range(CPQ):
        tk = np.arange(128)[:, None]
        tq = np.arange(QT)[None, :]
        cm[j * 128 : (j + 1) * 128] = (tq >= j * 128 + tk).astype(BF)

    def pack_mlp(wm):  # wm [MIDF, D] row-major -> [128, MFC, DC*128]
        a = (wm * n2_w[None, :]).T.astype(BF)  # [D, MIDF]
        a = a.reshape(D // 128, 128, MIDF // 128, 128)  # [c, p, mc, m]
        a = a.transpose(1, 2, 0, 3).reshape(128, MIDF // 128, (D // 128) * 128)
        return np.ascontiguousarray(a)

    wg_p = pack_mlp(np.asarray(wg_w))
    w1_p = pack_mlp(np.asarray(w1_w))
    w2_m = np.ascontiguousarray(np.asarray(w2_w).T.astype(BF))  # [MIDF, D]

    in_maps = []
    for c in range(8):
        b, r = c // 4, c % 4
        qs = slice(r * VF, (r + 1) * VF)
        qr = qkv_w[0 * D :][qs] * n1_w[None, :]
        kr = qkv_w[1 * D :][qs] * n1_w[None, :]
        vr = qkv_w[2 * D :][qs] * n1_w[None, :]
        in_maps.append(
            {
                "x": np.ascontiguousarray(x[b, :T], np.float32),
                "qkw_t": np.ascontiguousarray(
                    np.concatenate([qr, kr], 0).T.astype(BF)
                ),
                "vw_m": np.ascontiguousarray(vr.T.astype(BF)),
                "ow_m": np.ascontiguousarray(o_w[:, qs].T.astype(BF)),
                "w1w_p": w1_p,
                "wgw_p": wg_p,
                "w2w_m": w2_m,
                "cosr": cosr,
                "sinr": sinr,
                "cmask": cm,
                "ident": np.eye(128, dtype=BF),
            }
        )
    return in_maps


_CACHE = {}


def _get_nc(T):
    if T not in _CACHE:
        _CACHE[T] = build_nc(T)
    return _CACHE[T]


def run(inputs, T=2048, trace=False):
    nc = _get_nc(T)
    in_maps = make_in_maps(T=T, **inputs)
    res = run_bass_kernel_spmd(nc, in_maps, core_ids=list(range(8)), trace=trace)
    WT = T // 8  # 256
    out = np.empty((B, T, D), np.float32)
    for b in range(B):
        for r in range(4):
            o = res.results[4 * b + r]["out"]  # [512, D]
            out[b, r * WT : (r + 1) * WT] = o[0:WT]
            out[b, T // 2 + r * WT : T // 2 + (r + 1) * WT] = o[WT : 2 * WT]
    return out, res


def kernel(**inputs):
    out, _ = run(inputs, T=2048)
    return out
